# revision 20
# baseline (speedup 1.0000x reference)
"""Trainium2 Bass kernel for nn_MemAttention (SAM2-style memory attention).

Strategy (8 NeuronCores, sequence-parallel):
  - 4096 query tokens sharded 512/core. Residual stream kept TRANSPOSED
    xT [256(d) x 512(tok)] per core; the d axis is host-permuted to
    (evens || odds) so RoPE pairs are rows (i, i+128).
  - Per layer: self-attn K (roped) + V of the own slice are AllGathered
    (bf16) across the 8 cores; cross-attn memory keys are recomputed per
    core (projection + RoPE), and memory V is never materialized:
    U = exp(scores) @ [mem | 1] accumulates attention in the 64-dim
    memory space (last row = softmax denominator), then Wv up-projects.
  - LayerNorm runs in transposed space: partition-dim sums via
    ones-matmuls on fp32; per-token scale/shift broadcast via rank-1
    matmuls. 1/x and 1/sqrt(x) computed as Exp(-Ln(x)) / Exp(-0.5 Ln(x))
    so the ACT engine stays on one table set with the softmax Exp.
  - Residual stream + LN stats fp32; matmul operand path bf16.
"""
import os
import sys

sys.path.insert(0, "/opt/trn_rl_repo")

VARIANT = os.environ.get("KERNEL_VARIANT", "")
REPS = int(os.environ.get("KERNEL_REPS", "1"))

import numpy as np
import ml_dtypes

import concourse.bacc as bacc
import concourse.mybir as mybir
from concourse.tile import TileContext
from concourse.bass_utils import run_bass_kernel_spmd

D = 256
SEQ = 4096
L = 4
DFF = 2048
KV = 64
FEAT = 64
NCORE = 8
S = SEQ // NCORE          # 512 queries per core
NSP = 4 * SEQ             # 16384 spatial memory keys
NOBJ = 64                 # object-pointer keys (no rope)
NK = NSP + NOBJ
NSPC = NSP // 512         # 32 spatial 512-key chunks
THETA = 10000.0
EPS = 1e-5

F32 = mybir.dt.float32
F32R = mybir.dt.float32r
BF16 = mybir.dt.bfloat16
AL = mybir.AluOpType
AF = mybir.ActivationFunctionType
BF_NP = ml_dtypes.bfloat16

# d-permutation: new i = old 2i (i<128), new 128+i = old 2i+1
PERM = np.concatenate([np.arange(0, D, 2), np.arange(1, D, 2)])

# ----------------------------------------------------------------------------
# host-side prep
# ----------------------------------------------------------------------------


def _img(WT):
    """[K, M] (K multiple of 128) -> SBUF image [128, (K//128)*M]."""
    K, M = WT.shape
    kh = K // 128
    return np.ascontiguousarray(
        WT.reshape(kh, 128, M).transpose(1, 0, 2).reshape(128, kh * M)
    )


def _bimg(b):
    """[256] bias -> [128, 2] per-partition image (d-half on free dim)."""
    return np.ascontiguousarray(np.asarray(b, np.float32)[PERM].reshape(2, 128).T)


def _axial_tables():
    freqs = 1.0 / (THETA ** (np.arange(0, D, 4)[: D // 4].astype(np.float32) / D))
    t = np.arange(SEQ, dtype=np.float32)
    ang = np.concatenate(
        [np.outer(t % FEAT, freqs), np.outer(t // FEAT, freqs)], axis=-1
    )  # [4096, 128]
    return np.cos(ang).T.copy(), np.sin(ang).T.copy()  # [128, 4096]


def prepare_inputs(inp):
    """Build the 8 per-core in_maps from the full problem inputs."""
    f32 = np.float32
    feat = np.asarray(inp["current_vision_feat"], f32).reshape(D, SEQ)
    no_mem = np.asarray(inp["no_mem_embed"], f32).reshape(D, 1)
    pos = np.asarray(inp["current_vision_pos_embed"], f32)[:, 0].T  # [256,4096]
    xT0 = (feat - no_mem + 0.1 * pos)[PERM]  # [256, 4096]

    cosT, sinT = _axial_tables()

    mem_sp = (
        np.asarray(inp["memory_2"], f32)
        .reshape(-1, KV, SEQ)
        .transpose(0, 2, 1)
        .reshape(-1, KV)
    )
    obj = (
        np.asarray(inp["memory_1"], f32)
        .reshape(-1, 1, 4, KV)
        .transpose(0, 2, 1, 3)
        .reshape(-1, KV)
    )
    mem = np.concatenate([mem_sp, obj], axis=0)  # [16448, 64]
    mem_pos = np.concatenate(
        [
            np.asarray(inp["memory_pos_1"], f32)[:, 0],
            np.asarray(inp["memory_pos_2"], f32)[:, 0],
        ],
        axis=0,
    )
    memT_k = np.ascontiguousarray((mem + mem_pos).T)  # [64, 16448]
    mem_aug = np.concatenate([mem, np.ones((NK, 1), f32)], axis=1)  # [16448, 65]
    mem_aug = np.concatenate(
        [mem_aug, np.zeros((128 - NOBJ, KV + 1), f32)], axis=0
    )  # [16512, 65]
    mem_aug_img = _img(mem_aug)  # [128, 129*65]

    def wT(w, pin=False, pout=False):
        w = np.asarray(w, f32)
        if pout:
            w = w[PERM]
        if pin:
            w = w[:, PERM]
        return np.ascontiguousarray(w.T)

    st = {}
    bf = lambda a: np.ascontiguousarray(a).astype(BF_NP)
    st["w_sa_q"] = bf(np.stack([_img(wT(inp["sa_Wq"][l], True, True)) for l in range(L)]))
    st["w_sa_k"] = bf(np.stack([_img(wT(inp["sa_Wk"][l], True, True)) for l in range(L)]))
    st["w_sa_v"] = bf(np.stack([_img(wT(inp["sa_Wv"][l], True, True)) for l in range(L)]))
    st["w_sa_o"] = bf(np.stack([_img(wT(inp["sa_Wo"][l], True, True)) for l in range(L)]))
    st["w_ca_q"] = bf(np.stack([_img(wT(inp["ca_Wq"][l], True, True)) for l in range(L)]))
    # ca_Wk/ca_Wv: [256, 64]; in-dim 64 (mem space, unpermuted), out-dim permuted
    st["w_ca_k"] = bf(np.stack([wT(inp["ca_Wk"][l], False, True) for l in range(L)]))
    st["w_ca_v"] = bf(np.stack([wT(inp["ca_Wv"][l], False, True) for l in range(L)]))
    st["w_ca_o"] = bf(np.stack([_img(wT(inp["ca_Wo"][l], True, True)) for l in range(L)]))
    st["w_ff1"] = bf(np.stack([_img(wT(inp["ff_W1"][l], pin=True)) for l in range(L)]))
    st["w_ff2"] = bf(np.stack([_img(wT(inp["ff_W2"][l], pout=True)) for l in range(L)]))

    st["b_sa_q"] = np.stack([_bimg(inp["sa_bq"][l]) for l in range(L)])
    st["b_sa_k"] = np.stack([_bimg(inp["sa_bk"][l]) for l in range(L)])
    st["b_ca_q"] = np.stack([_bimg(inp["ca_bq"][l]) for l in range(L)])
    st["b_ca_k"] = np.stack([_bimg(inp["ca_bk"][l]) for l in range(L)])
    st["b_ca_k_row"] = np.stack(
        [np.asarray(inp["ca_bk"][l], np.float32)[PERM].reshape(1, 256) for l in range(L)]
    )
    st["b_sa_o"] = np.stack(
        [
            _bimg(
                np.asarray(inp["sa_bo"][l], f32)
                + np.asarray(inp["sa_Wo"][l], f32) @ np.asarray(inp["sa_bv"][l], f32)
            )
            for l in range(L)
        ]
    )
    st["b_ca_o"] = np.stack(
        [
            _bimg(
                np.asarray(inp["ca_bo"][l], f32)
                + np.asarray(inp["ca_Wo"][l], f32) @ np.asarray(inp["ca_bv"][l], f32)
            )
            for l in range(L)
        ]
    )
    st["b_ff1"] = np.stack(
        [np.asarray(inp["ff_b1"][l], f32).reshape(16, 128).T for l in range(L)]
    )
    st["b_ff2"] = np.stack([_bimg(inp["ff_b2"][l]) for l in range(L)])

    def ln_img(s_, b_):
        g = np.asarray(s_, f32)[PERM].reshape(2, 128).T   # [128, 2]
        bb = np.asarray(b_, f32)[PERM].reshape(2, 128).T  # [128, 2]
        return np.concatenate([g, bb], axis=1)  # [128, 4] = (gam dh0, gam dh1, b dh0, b dh1)

    ln_all = np.zeros((L, 128, 12), f32)
    for l in range(L):
        ln_all[l, :, 0:4] = ln_img(inp["n1_s"][l], inp["n1_b"][l])
        ln_all[l, :, 4:8] = ln_img(inp["n2_s"][l], inp["n2_b"][l])
        ln_all[l, :, 8:12] = ln_img(inp["n3_s"][l], inp["n3_b"][l])
    st["ln_all"] = np.ascontiguousarray(ln_all)
    st["ln_fn"] = np.ascontiguousarray(ln_img(inp["fn_s"], inp["fn_b"]))

    shared = {
        "cos_k": bf(cosT),
        "sin_k": bf(sinT),
        "memT_k": bf(memT_k),
        "mem_aug": bf(mem_aug_img),
        **{k: np.ascontiguousarray(v) for k, v in st.items()},
    }
    in_maps = []
    for c in range(NCORE):
        m = dict(shared)
        m["xT0"] = np.ascontiguousarray(xT0[:, c * S : (c + 1) * S])
        m["cos_q"] = bf(cosT[:, c * S : (c + 1) * S])
        m["sin_q"] = bf(sinT[:, c * S : (c + 1) * S])
        in_maps.append(m)
    return in_maps


# ----------------------------------------------------------------------------
# device kernel
# ----------------------------------------------------------------------------


def build_nc():
    nc = bacc.Bacc("TRN2", target_bir_lowering=False, debug=False, num_devices=NCORE)

    def din(name, shape, dt=F32):
        return nc.dram_tensor(name, list(shape), dt, kind="ExternalInput").ap()

    T = {}
    T["xT0"] = din("xT0", [D, S])
    T["cos_q"] = din("cos_q", [128, S], BF16)
    T["sin_q"] = din("sin_q", [128, S], BF16)
    T["cos_k"] = din("cos_k", [128, SEQ], BF16)
    T["sin_k"] = din("sin_k", [128, SEQ], BF16)
    T["memT_k"] = din("memT_k", [KV, NK], BF16)
    T["mem_aug"] = din("mem_aug", [128, 129 * (KV + 1)], BF16)
    for n in ["w_sa_q", "w_sa_k", "w_sa_v", "w_sa_o", "w_ca_q", "w_ca_o"]:
        T[n] = din(n, [L, 128, 512], BF16)
    T["w_ca_k"] = din("w_ca_k", [L, KV, 256], BF16)
    T["w_ca_v"] = din("w_ca_v", [L, KV, 256], BF16)
    T["w_ff1"] = din("w_ff1", [L, 128, 4096], BF16)
    T["w_ff2"] = din("w_ff2", [L, 128, 4096], BF16)
    for n in ["b_sa_q", "b_sa_k", "b_ca_q", "b_ca_k", "b_sa_o", "b_ca_o", "b_ff2"]:
        T[n] = din(n, [L, 128, 2])
    T["b_ff1"] = din("b_ff1", [L, 128, 16])
    T["b_ca_k_row"] = din("b_ca_k_row", [L, 1, 256])
    T["ln_all"] = din("ln_all", [L, 128, 12])
    T["ln_fn"] = din("ln_fn", [128, 4])

    T["out"] = nc.dram_tensor("outT", [D, S], F32, kind="ExternalOutput").ap()

    CCN = 2 * D * S  # bf16 elements per rank: k [256*512] + v [512*256]
    T["cc_in"] = nc.dram_tensor("cc_in", [CCN], BF16, kind="Internal").ap()
    T["cc_out"] = nc.dram_tensor(
        "cc_out", [NCORE * CCN], BF16, kind="Internal", addr_space="Shared"
    ).ap()
    if "nocc" in VARIANT:
        T["cc_out"] = nc.dram_tensor("cc_loc", [NCORE * CCN], BF16, kind="Internal").ap()

    with TileContext(nc) as tc:
        _emit(nc, tc, T)
    nc.compile()
    return nc


def _emit(nc, tc, T):
    import contextlib

    ctx = contextlib.ExitStack()
    const = ctx.enter_context(tc.tile_pool(name="const", bufs=1))
    wpool = ctx.enter_context(tc.tile_pool(name="wpool", bufs=2))
    act = ctx.enter_context(tc.tile_pool(name="act", bufs=2))
    small = ctx.enter_context(tc.tile_pool(name="small", bufs=4))
    rows = ctx.enter_context(tc.tile_pool(name="rows", bufs=8))
    pmm = ctx.enter_context(tc.tile_pool(name="pmm", bufs=2, space="PSUM"))
    pacc = ctx.enter_context(tc.tile_pool(name="pacc", bufs=2, space="PSUM"))
    prow = pmm

    V = nc.vector
    SC = nc.scalar
    TE = nc.tensor
    SY = nc.sync

    cosq = const.tile([128, S], BF16)
    sinq = const.tile([128, S], BF16)
    maug = const.tile([128, 129 * (KV + 1)], BF16)
    cosk = const.tile([128, SEQ], BF16)
    sink = const.tile([128, SEQ], BF16)
    SY.dma_start(out=cosq, in_=T["cos_q"])
    SY.dma_start(out=sinq, in_=T["sin_q"])
    SY.dma_start(out=cosk, in_=T["cos_k"])
    SY.dma_start(out=sink, in_=T["sin_k"])
    SY.dma_start(out=maug, in_=T["mem_aug"])
    ones_col_bf = const.tile([128, 1], BF16)
    V.memset(ones_col_bf, 1.0)
    ones_col_f = const.tile([128, 1], F32)
    V.memset(ones_col_f, 1.0)
    ones_col = const.tile([128, 1], F32R)
    V.tensor_copy(ones_col, ones_col_f)
    ones_row_f = const.tile([1, S], F32)
    V.memset(ones_row_f, 1.0)
    ones_row = const.tile([1, 128], F32R)
    V.tensor_copy(ones_row, ones_row_f[:, 0:128])
    ones_row_s = const.tile([1, S], F32R)
    V.tensor_copy(ones_row_s, ones_row_f)
    eps_row = const.tile([1, 1], F32)
    V.memset(eps_row, EPS)
    lnfn = const.tile([128, 4], F32)
    SY.dma_start(out=lnfn, in_=T["ln_fn"])

    x = act.tile([128, 2 * S], F32R, tag="x")
    SY.dma_start(out=x.rearrange("p (dh s) -> p dh s", dh=2),
                 in_=T["xT0"].bitcast(F32R).rearrange("(dh p) s -> p dh s", p=128))

    def f32r(ap):
        return ap.bitcast(F32R)

    def layer_norm(x_in, ln_tile, ln_off, out_dt=BF16):
        s1 = prow.tile([1, S], F32, tag="mm")
        xsq = act.tile([128, 2 * S], F32R, tag="xsq", bufs=1)
        SC.activation(out=xsq, in_=x_in, func=AF.Square)
        for dh in range(2):
            TE.matmul(s1, ones_col, x_in[:, dh * S : (dh + 1) * S],
                      start=(dh == 0), stop=(dh == 1))
        s2 = prow.tile([1, S], F32, tag="mm")
        for dh in range(2):
            TE.matmul(s2, ones_col, xsq[:, dh * S : (dh + 1) * S],
                      start=(dh == 0), stop=(dh == 1))
        m = rows.tile([1, S], F32R, tag="r")
        V.tensor_scalar_mul(m, s1, 1.0 / D)
        msq = rows.tile([1, S], F32, tag="r")
        V.tensor_tensor(msq, m, m, AL.mult)
        vr = rows.tile([1, S], F32, tag="r")
        V.scalar_tensor_tensor(out=vr, in0=s2, scalar=1.0 / D, in1=msq,
                               op0=AL.mult, op1=AL.subtract)
        lnv = rows.tile([1, S], F32, tag="r")
        SC.activation(out=lnv, in_=vr, func=AF.Ln, bias=eps_row)
        rstd = rows.tile([1, S], F32R, tag="r")
        SC.activation(out=rstd, in_=lnv, func=AF.Exp, scale=-0.5)

        bm_ps = pmm.tile([128, S], F32, tag="mm")
        TE.matmul(bm_ps, ones_row, m, start=True, stop=True)
        a_ps = pmm.tile([128, S], F32, tag="mm")
        TE.matmul(a_ps, ones_row, rstd, start=True, stop=True)
        t2 = act.tile([128, 2 * S], out_dt, tag="t2")
        for dh in range(2):
            u1 = small.tile([128, S], F32, tag="lntmp")
            V.tensor_tensor(u1, x_in[:, dh * S : (dh + 1) * S], bm_ps, AL.subtract)
            u2 = small.tile([128, S], F32, tag="lntmp")
            V.tensor_tensor(u2, u1, a_ps, AL.mult)
            V.tensor_scalar(out=t2[:, dh * S : (dh + 1) * S], in0=u2,
                            scalar1=ln_tile[:, ln_off + dh : ln_off + dh + 1],
                            scalar2=ln_tile[:, ln_off + 2 + dh : ln_off + 3 + dh],
                            op0=AL.mult, op1=AL.add)
        return t2

    GP = nc.gpsimd

    def rope(src_bf, cos_ap, sin_ap, dst_bf, n=S):
        e = src_bf[:, 0:n]
        o = src_bf[:, n : 2 * n]
        t1 = small.tile([128, S], BF16, tag="rtv")
        t2_ = small.tile([128, S], BF16, tag="rtv")
        V.tensor_tensor(t1[:, 0:n], e, cos_ap, AL.mult)
        V.tensor_tensor(t2_[:, 0:n], o, sin_ap, AL.mult)
        V.tensor_tensor(dst_bf[:, 0:n], t1[:, 0:n], t2_[:, 0:n], AL.subtract)
        t3 = small.tile([128, S], BF16, tag="rtv")
        t4 = small.tile([128, S], BF16, tag="rtv")
        V.tensor_tensor(t3[:, 0:n], e, sin_ap, AL.mult)
        V.tensor_tensor(t4[:, 0:n], o, cos_ap, AL.mult)
        V.tensor_tensor(dst_bf[:, n : 2 * n], t3[:, 0:n], t4[:, 0:n], AL.add)

    def proj_qk(t2, w_tile, b_tile, dst_bf):
        for dh in range(2):
            ps = pmm.tile([128, S], F32, tag="mm")
            for kh in range(2):
                TE.matmul(ps, w_tile[:, kh * 256 + dh * 128 : kh * 256 + dh * 128 + 128],
                          t2[:, kh * S : (kh + 1) * S],
                          start=(kh == 0), stop=(kh == 1))
            V.tensor_scalar_add(dst_bf[:, dh * S : (dh + 1) * S], ps,
                                b_tile[:, dh : dh + 1])

    def load_w(name, l, shape, dt=F32, bufs=None):
        t = wpool.tile(shape, dt, tag=name, bufs=bufs)
        SY.dma_start(out=t, in_=T[name][l])
        return t

    for l in [ll for _ in range(REPS) for ll in range(L)]:
        wsaq = load_w("w_sa_q", l, [128, 512], BF16)
        wsak = load_w("w_sa_k", l, [128, 512], BF16)
        wsav = load_w("w_sa_v", l, [128, 512], BF16)
        wsao = load_w("w_sa_o", l, [128, 512], BF16)
        wcaq = load_w("w_ca_q", l, [128, 512], BF16)
        wcak = load_w("w_ca_k", l, [KV, 256], BF16)
        wcav = load_w("w_ca_v", l, [KV, 256], BF16)
        wcao = load_w("w_ca_o", l, [128, 512], BF16)
        wff1 = load_w("w_ff1", l, [128, 4096], BF16, bufs=1)
        wff2 = load_w("w_ff2", l, [128, 4096], BF16, bufs=1)
        bsaq = load_w("b_sa_q", l, [128, 2])
        bsak = load_w("b_sa_k", l, [128, 2])
        bcaq = load_w("b_ca_q", l, [128, 2])
        bcak = load_w("b_ca_k", l, [128, 2])
        bsao = load_w("b_sa_o", l, [128, 2])
        bcao = load_w("b_ca_o", l, [128, 2])
        bff1 = load_w("b_ff1", l, [128, 16])
        bff2 = load_w("b_ff2", l, [128, 2])
        lnw = load_w("ln_all", l, [128, 12])

        # ---------- self attention ----------
        t2 = layer_norm(x, lnw, 0)
        qpre = act.tile([128, 2 * S], BF16, tag="qpre")
        proj_qk(t2, wsaq, bsaq, qpre)
        qr = act.tile([128, 2 * S], BF16, tag="qr")
        rope(qpre, cosq, sinq, qr)
        kpre = act.tile([128, 2 * S], BF16, tag="qpre")
        proj_qk(t2, wsak, bsak, kpre)
        kr = act.tile([128, 2 * S], BF16, tag="kr")
        rope(kpre, cosq, sinq, kr)

        vown = act.tile([128, 4 * 256], BF16, tag="vown")
        for tcb in range(4):
            ps = pmm.tile([128, 256], F32, tag="mm")
            for kh in range(2):
                TE.matmul(ps,
                          t2[:, kh * S + tcb * 128 : kh * S + tcb * 128 + 128],
                          wsav[:, kh * 256 : (kh + 1) * 256],
                          start=(kh == 0), stop=(kh == 1))
            V.tensor_copy(vown[:, tcb * 256 : (tcb + 1) * 256], ps)

        SY.dma_start(
            out=T["cc_in"][0 : D * S].rearrange("(dh p s) -> p dh s", p=128, s=S),
            in_=kr.rearrange("p (dh s) -> p dh s", dh=2),
        )
        SY.dma_start(
            out=T["cc_in"][D * S : 2 * D * S].rearrange(
                "(tc p o) -> p tc o", p=128, o=256
            ),
            in_=vown.rearrange("p (tc o) -> p tc o", o=256),
        )
        if "nocc" in VARIANT:
            # timing bisection only: fake the gather with local copies
            for r in range(NCORE):
                SY.dma_start(out=T["cc_out"][r * 2 * D * S : (r + 1) * 2 * D * S],
                             in_=T["cc_in"][:])
        else:
            nc.gpsimd.collective_compute(
                "AllGather",
                AL.bypass,
                replica_groups=[list(range(NCORE))],
                ins=[T["cc_in"][:]],
                outs=[T["cc_out"][:]],
            )
        av0 = pacc.tile([128, S], F32, tag="acc")
        av1 = pacc.tile([128, S], F32, tag="acc")
        den = prow.tile([1, S], F32, tag="mm")
        for kc2 in range(16):
            sc2 = pmm.tile([128, 2 * S], F32, tag="mm2", bufs=2)
            P2 = small.tile([128, 2 * S], BF16, tag="P", bufs=4)
            kchs = []
            vchs = []
            for h in range(2):
                kc = 2 * kc2 + h
                r_, tcq = kc // 4, kc % 4
                base = r_ * 2 * D * S
                kch = small.tile([128, 256], BF16, tag="kch", bufs=4)
                SY.dma_start(
                    out=kch.rearrange("p (dh c) -> p dh c", dh=2),
                    in_=T["cc_out"][base : base + D * S].rearrange(
                        "(dh p s) -> p dh s", p=128, s=S
                    )[:, :, tcq * 128 : (tcq + 1) * 128],
                )
                vch = small.tile([128, 256], BF16, tag="vch", bufs=4)
                SY.dma_start(
                    out=vch,
                    in_=T["cc_out"][
                        base + D * S + tcq * 128 * 256 : base + D * S
                        + (tcq + 1) * 128 * 256
                    ].rearrange("(p o) -> p o", p=128),
                )
                kchs.append(kch)
                vchs.append(vch)
                for dh in range(2):
                    TE.matmul(sc2[:, h * S : (h + 1) * S],
                              kch[:, dh * 128 : (dh + 1) * 128],
                              qr[:, dh * S : (dh + 1) * S],
                              start=(dh == 0), stop=(dh == 1))
            SC.activation(out=P2, in_=sc2, func=AF.Exp, scale=1.0 / 16.0)
            for h in range(2):
                kc = 2 * kc2 + h
                Ph = P2[:, h * S : (h + 1) * S]
                TE.matmul(den, ones_col_bf, Ph, start=(kc == 0), stop=(kc == 31))
                TE.matmul(av0, vchs[h][:, 0:128], Ph,
                          start=(kc == 0), stop=(kc == 31))
                TE.matmul(av1, vchs[h][:, 128:256], Ph,
                          start=(kc == 0), stop=(kc == 31))

        lnden = rows.tile([1, S], F32, tag="r")
        SC.activation(out=lnden, in_=den, func=AF.Ln)
        recip = rows.tile([1, S], F32R, tag="r")
        SC.activation(out=recip, in_=lnden, func=AF.Exp, scale=-1.0)
        bc = pmm.tile([128, S], F32, tag="mm")
        TE.matmul(bc, ones_row, recip, start=True, stop=True)
        bcs = small.tile([128, S], F32, tag="bcs", bufs=2)
        V.tensor_copy(bcs, bc)
        oin = act.tile([128, 2 * S], BF16, tag="oin")
        V.tensor_tensor(oin[:, 0:S], av0, bcs, AL.mult)
        V.tensor_tensor(oin[:, S : 2 * S], av1, bcs, AL.mult)

        xn = act.tile([128, 2 * S], F32R, tag="x")
        for dh in range(2):
            ps = pmm.tile([128, S], F32, tag="mm")
            for kh in range(2):
                TE.matmul(ps, wsao[:, kh * 256 + dh * 128 : kh * 256 + dh * 128 + 128],
                          oin[:, kh * S : (kh + 1) * S],
                          start=(kh == 0), stop=(kh == 1))
            V.scalar_tensor_tensor(out=xn[:, dh * S : (dh + 1) * S], in0=ps,
                                   scalar=bsao[:, dh : dh + 1],
                                   in1=x[:, dh * S : (dh + 1) * S],
                                   op0=AL.add, op1=AL.add)
        x = xn

        # ---------- cross attention ----------
        t2c = layer_norm(x, lnw, 4)
        qpre2 = act.tile([128, 2 * S], BF16, tag="qpre")
        proj_qk(t2c, wcaq, bcaq, qpre2)
        qrc = act.tile([128, 2 * S], BF16, tag="qr")
        rope(qpre2, cosq, sinq, qrc)

        U = pacc.tile([128, S], F32, tag="acc")
        n_mm_u = NSPC * 4 + 1
        mmu = 0
        for scc in range(NSPC):
            koff = scc * 512
            coff = (scc % 8) * 512
            mkc = small.tile([KV, 512], BF16, tag="mkc", bufs=3)
            SY.dma_start(out=mkc, in_=T["memT_k"][:, koff : koff + 512])
            kme = small.tile([128, 2 * 512], BF16, tag="kme", bufs=3)
            for dh in range(2):
                kp = pmm.tile([128, 512], F32, tag="mm")
                TE.matmul(kp, wcak[:, dh * 128 : (dh + 1) * 128], mkc,
                          start=True, stop=True)
                V.tensor_scalar_add(kme[:, dh * 512 : (dh + 1) * 512], kp,
                                    bcak[:, dh : dh + 1])
            krm = small.tile([128, 2 * 512], BF16, tag="krm", bufs=4)
            rope(kme, cosk[:, coff : coff + 512], sink[:, coff : coff + 512],
                 krm, n=512)
            for jp in range(2):
                sc2 = pmm.tile([128, 2 * S], F32, tag="mm2", bufs=2)
                P2 = small.tile([128, 2 * S], BF16, tag="P", bufs=4)
                for h in range(2):
                    j = 2 * jp + h
                    for dh in range(2):
                        TE.matmul(sc2[:, h * S : (h + 1) * S],
                                  krm[:, dh * 512 + j * 128 : dh * 512 + j * 128 + 128],
                                  qrc[:, dh * S : (dh + 1) * S],
                                  start=(dh == 0), stop=(dh == 1))
                SC.activation(out=P2, in_=sc2, func=AF.Exp, scale=1.0 / 16.0)
                for h in range(2):
                    ci = scc * 4 + 2 * jp + h
                    TE.matmul(U[0:65, :], maug[:, ci * 65 : (ci + 1) * 65],
                              P2[:, h * S : (h + 1) * S],
                              start=(mmu == 0), stop=(mmu == n_mm_u - 1))
                    mmu += 1
        # object pointer chunk (64 keys, no rope)
        mko = small.tile([KV, 64], BF16, tag="mko", bufs=1)
        SY.dma_start(out=mko, in_=T["memT_k"][:, NSP:NK])
        kobj = small.tile([128, 2 * 64], BF16, tag="kobj", bufs=1)
        for dh in range(2):
            kp = pmm.tile([128, 64], F32, tag="mm")
            TE.matmul(kp, wcak[:, dh * 128 : (dh + 1) * 128], mko,
                      start=True, stop=True)
            V.tensor_scalar_add(kobj[:, dh * 64 : (dh + 1) * 64], kp,
                                bcak[:, dh : dh + 1])
        sco = pmm.tile([64, S], F32, tag="mm")
        for dh in range(2):
            TE.matmul(sco, kobj[:, dh * 64 : (dh + 1) * 64],
                      qrc[:, dh * S : (dh + 1) * S],
                      start=(dh == 0), stop=(dh == 1))
        Po = small.tile([64, S], BF16, tag="Po", bufs=1)
        SC.activation(out=Po, in_=sco, func=AF.Exp, scale=1.0 / 16.0)
        TE.matmul(U[0:65, :], maug[0:64, 128 * 65 : 128 * 65 + 65], Po,
                  start=False, stop=True)

        lnd2 = rows.tile([1, S], F32, tag="r")
        SC.activation(out=lnd2, in_=U[64:65, :], func=AF.Ln)
        rec2 = rows.tile([1, S], F32R, tag="r")
        SC.activation(out=rec2, in_=lnd2, func=AF.Exp, scale=-1.0)
        bc2 = pmm.tile([128, S], F32, tag="mm")
        TE.matmul(bc2, ones_row, rec2, start=True, stop=True)
        bcs2 = small.tile([128, S], F32, tag="bcs", bufs=2)
        V.tensor_copy(bcs2, bc2)
        un = small.tile([64, S], BF16, tag="un", bufs=2)
        V.tensor_tensor(un, U[0:64, :], bcs2[0:64, :], AL.mult)

        oin2 = act.tile([128, 2 * S], BF16, tag="oin")
        for dh in range(2):
            ps = pmm.tile([128, S], F32, tag="mm")
            TE.matmul(ps, wcav[:, dh * 128 : (dh + 1) * 128], un,
                      start=True, stop=True)
            V.tensor_copy(oin2[:, dh * S : (dh + 1) * S], ps)
        xn2 = act.tile([128, 2 * S], F32R, tag="x")
        for dh in range(2):
            ps = pmm.tile([128, S], F32, tag="mm")
            for kh in range(2):
                TE.matmul(ps, wcao[:, kh * 256 + dh * 128 : kh * 256 + dh * 128 + 128],
                          oin2[:, kh * S : (kh + 1) * S],
                          start=(kh == 0), stop=(kh == 1))
            V.scalar_tensor_tensor(out=xn2[:, dh * S : (dh + 1) * S], in0=ps,
                                   scalar=bcao[:, dh : dh + 1],
                                   in1=x[:, dh * S : (dh + 1) * S],
                                   op0=AL.add, op1=AL.add)
        x = xn2

        # ---------- FFN ----------
        t3 = layer_norm(x, lnw, 8)
        hT = act.tile([128, 16 * S], BF16, tag="hT", bufs=1)
        for mc in range(16):
            ps = pmm.tile([128, S], F32, tag="mm")
            for kh in range(2):
                TE.matmul(ps,
                          wff1[:, kh * 2048 + mc * 128 : kh * 2048 + mc * 128 + 128],
                          t3[:, kh * S : (kh + 1) * S],
                          start=(kh == 0), stop=(kh == 1))
            V.tensor_scalar(out=hT[:, mc * S : (mc + 1) * S], in0=ps,
                            scalar1=bff1[:, mc : mc + 1], scalar2=0.0,
                            op0=AL.add, op1=AL.max)
        xn3 = act.tile([128, 2 * S], F32R, tag="x")
        for dh in range(2):
            ps = pmm.tile([128, S], F32, tag="mm")
            for kh in range(16):
                TE.matmul(ps, wff2[:, kh * 256 + dh * 128 : kh * 256 + dh * 128 + 128],
                          hT[:, kh * S : (kh + 1) * S],
                          start=(kh == 0), stop=(kh == 15))
            V.scalar_tensor_tensor(out=xn3[:, dh * S : (dh + 1) * S], in0=ps,
                                   scalar=bff2[:, dh : dh + 1],
                                   in1=x[:, dh * S : (dh + 1) * S],
                                   op0=AL.add, op1=AL.add)
        x = xn3

    xf = layer_norm(x, lnfn, 0, out_dt=F32)
    SY.dma_start(out=T["out"].rearrange("(dh p) s -> p dh s", p=128),
                 in_=xf.rearrange("p (dh s) -> p dh s", dh=2))
    ctx.close()


# ----------------------------------------------------------------------------
# entry point
# ----------------------------------------------------------------------------

_NC_CACHE = {}


def _get_nc():
    if "nc" not in _NC_CACHE:
        _NC_CACHE["nc"] = build_nc()
    return _NC_CACHE["nc"]


def kernel(**inputs):
    nc = _get_nc()
    in_maps = prepare_inputs(inputs)
    res = run_bass_kernel_spmd(nc, in_maps, core_ids=list(range(NCORE)))
    outs = [np.asarray(res.results[c]["outT"]) for c in range(NCORE)]
    X = np.concatenate(outs, axis=1)  # [256, 4096] in permuted-d space
    Xo = np.empty_like(X)
    Xo[PERM] = X
    return np.ascontiguousarray(Xo.reshape(1, D, FEAT, FEAT)).astype(np.float32)


# revision 22
# speedup vs baseline: 1.7055x; 1.7055x over previous
"""Trainium2 Bass kernel for nn_MemAttention (SAM2-style memory attention).

Strategy (8 NeuronCores, sequence-parallel):
  - 4096 query tokens sharded 512/core. Residual stream kept TRANSPOSED
    xT [256(d) x 512(tok)] per core; the d axis is host-permuted to
    (evens || odds) so RoPE pairs are rows (i, i+128).
  - Per layer: self-attn K (roped) + V of the own slice are AllGathered
    (bf16) across the 8 cores; cross-attn memory keys are recomputed per
    core (projection + RoPE), and memory V is never materialized:
    U = exp(scores) @ [mem | 1] accumulates attention in the 64-dim
    memory space (last row = softmax denominator), then Wv up-projects.
  - LayerNorm runs in transposed space: partition-dim sums via
    ones-matmuls on fp32; per-token scale/shift broadcast via rank-1
    matmuls. 1/x and 1/sqrt(x) computed as Exp(-Ln(x)) / Exp(-0.5 Ln(x))
    so the ACT engine stays on one table set with the softmax Exp.
  - Residual stream + LN stats fp32; matmul operand path bf16.
"""
import os
import sys

sys.path.insert(0, "/opt/trn_rl_repo")

VARIANT = os.environ.get("KERNEL_VARIANT", "")
REPS = int(os.environ.get("KERNEL_REPS", "1"))

import numpy as np
import ml_dtypes

import concourse.bacc as bacc
import concourse.mybir as mybir
from concourse.tile import TileContext
from concourse.bass_utils import run_bass_kernel_spmd

D = 256
SEQ = 4096
L = 4
DFF = 2048
KV = 64
FEAT = 64
NCORE = 8
S = SEQ // NCORE          # 512 queries per core
NSP = 4 * SEQ             # 16384 spatial memory keys
NOBJ = 64                 # object-pointer keys (no rope)
NK = NSP + NOBJ
NSPC = NSP // 512         # 32 spatial 512-key chunks
THETA = 10000.0
EPS = 1e-5

F32 = mybir.dt.float32
F32R = mybir.dt.float32r
BF16 = mybir.dt.bfloat16
AL = mybir.AluOpType
AF = mybir.ActivationFunctionType
BF_NP = ml_dtypes.bfloat16

# d-permutation: new i = old 2i (i<128), new 128+i = old 2i+1
PERM = np.concatenate([np.arange(0, D, 2), np.arange(1, D, 2)])

# ----------------------------------------------------------------------------
# host-side prep
# ----------------------------------------------------------------------------


def _img(WT):
    """[K, M] (K multiple of 128) -> SBUF image [128, (K//128)*M]."""
    K, M = WT.shape
    kh = K // 128
    return np.ascontiguousarray(
        WT.reshape(kh, 128, M).transpose(1, 0, 2).reshape(128, kh * M)
    )


def _bimg(b):
    """[256] bias -> [128, 2] per-partition image (d-half on free dim)."""
    return np.ascontiguousarray(np.asarray(b, np.float32)[PERM].reshape(2, 128).T)


def _axial_tables():
    freqs = 1.0 / (THETA ** (np.arange(0, D, 4)[: D // 4].astype(np.float32) / D))
    t = np.arange(SEQ, dtype=np.float32)
    ang = np.concatenate(
        [np.outer(t % FEAT, freqs), np.outer(t // FEAT, freqs)], axis=-1
    )  # [4096, 128]
    return np.cos(ang).T.copy(), np.sin(ang).T.copy()  # [128, 4096]


def prepare_inputs(inp):
    """Build the 8 per-core in_maps from the full problem inputs."""
    f32 = np.float32
    feat = np.asarray(inp["current_vision_feat"], f32).reshape(D, SEQ)
    no_mem = np.asarray(inp["no_mem_embed"], f32).reshape(D, 1)
    pos = np.asarray(inp["current_vision_pos_embed"], f32)[:, 0].T  # [256,4096]
    xT0 = (feat - no_mem + 0.1 * pos)[PERM]  # [256, 4096]

    cosT, sinT = _axial_tables()

    mem_sp = (
        np.asarray(inp["memory_2"], f32)
        .reshape(-1, KV, SEQ)
        .transpose(0, 2, 1)
        .reshape(-1, KV)
    )
    obj = (
        np.asarray(inp["memory_1"], f32)
        .reshape(-1, 1, 4, KV)
        .transpose(0, 2, 1, 3)
        .reshape(-1, KV)
    )
    mem = np.concatenate([mem_sp, obj], axis=0)  # [16448, 64]
    mem_pos = np.concatenate(
        [
            np.asarray(inp["memory_pos_1"], f32)[:, 0],
            np.asarray(inp["memory_pos_2"], f32)[:, 0],
        ],
        axis=0,
    )
    memT_k = np.ascontiguousarray((mem + mem_pos).T)  # [64, 16448]
    mem_aug = np.concatenate([mem, np.ones((NK, 1), f32)], axis=1)  # [16448, 65]
    mem_aug = np.concatenate(
        [mem_aug, np.zeros((128 - NOBJ, KV + 1), f32)], axis=0
    )  # [16512, 65]
    mem_aug_img = _img(mem_aug)  # [128, 129*65]

    def wT(w, pin=False, pout=False):
        w = np.asarray(w, f32)
        if pout:
            w = w[PERM]
        if pin:
            w = w[:, PERM]
        return np.ascontiguousarray(w.T)

    st = {}
    bf = lambda a: np.ascontiguousarray(a).astype(BF_NP)
    st["w_sa_q"] = bf(np.stack([_img(wT(inp["sa_Wq"][l], True, True)) for l in range(L)]))
    st["w_sa_k"] = bf(np.stack([_img(wT(inp["sa_Wk"][l], True, True)) for l in range(L)]))
    st["w_sa_v"] = bf(np.stack([_img(wT(inp["sa_Wv"][l], True, True)) for l in range(L)]))
    st["w_sa_o"] = bf(np.stack([_img(wT(inp["sa_Wo"][l], True, True)) for l in range(L)]))
    st["w_ca_q"] = bf(np.stack([_img(wT(inp["ca_Wq"][l], True, True)) for l in range(L)]))
    # ca_Wk/ca_Wv: [256, 64]; in-dim 64 (mem space, unpermuted), out-dim permuted
    st["w_ca_k"] = bf(np.stack([wT(inp["ca_Wk"][l], False, True) for l in range(L)]))
    st["w_ca_v"] = bf(np.stack([wT(inp["ca_Wv"][l], False, True) for l in range(L)]))
    st["w_ca_o"] = bf(np.stack([_img(wT(inp["ca_Wo"][l], True, True)) for l in range(L)]))
    st["w_ff1"] = bf(np.stack([_img(wT(inp["ff_W1"][l], pin=True)) for l in range(L)]))
    st["w_ff2"] = bf(np.stack([_img(wT(inp["ff_W2"][l], pout=True)) for l in range(L)]))

    st["b_sa_q"] = np.stack([_bimg(inp["sa_bq"][l]) for l in range(L)])
    st["b_sa_k"] = np.stack([_bimg(inp["sa_bk"][l]) for l in range(L)])
    st["b_ca_q"] = np.stack([_bimg(inp["ca_bq"][l]) for l in range(L)])
    st["b_ca_k"] = np.stack([_bimg(inp["ca_bk"][l]) for l in range(L)])
    st["b_ca_k_row"] = np.stack(
        [np.asarray(inp["ca_bk"][l], np.float32)[PERM].reshape(1, 256) for l in range(L)]
    )
    st["b_sa_o"] = np.stack(
        [
            _bimg(
                np.asarray(inp["sa_bo"][l], f32)
                + np.asarray(inp["sa_Wo"][l], f32) @ np.asarray(inp["sa_bv"][l], f32)
            )
            for l in range(L)
        ]
    )
    st["b_ca_o"] = np.stack(
        [
            _bimg(
                np.asarray(inp["ca_bo"][l], f32)
                + np.asarray(inp["ca_Wo"][l], f32) @ np.asarray(inp["ca_bv"][l], f32)
            )
            for l in range(L)
        ]
    )
    st["b_ff1"] = np.stack(
        [np.asarray(inp["ff_b1"][l], f32).reshape(16, 128).T for l in range(L)]
    )
    st["b_ff2"] = np.stack([_bimg(inp["ff_b2"][l]) for l in range(L)])

    def ln_img(s_, b_):
        g = np.asarray(s_, f32)[PERM].reshape(2, 128).T   # [128, 2]
        bb = np.asarray(b_, f32)[PERM].reshape(2, 128).T  # [128, 2]
        return np.concatenate([g, bb], axis=1)  # [128, 4] = (gam dh0, gam dh1, b dh0, b dh1)

    ln_all = np.zeros((L, 128, 12), f32)
    for l in range(L):
        ln_all[l, :, 0:4] = ln_img(inp["n1_s"][l], inp["n1_b"][l])
        ln_all[l, :, 4:8] = ln_img(inp["n2_s"][l], inp["n2_b"][l])
        ln_all[l, :, 8:12] = ln_img(inp["n3_s"][l], inp["n3_b"][l])
    st["ln_all"] = np.ascontiguousarray(ln_all)
    st["ln_fn"] = np.ascontiguousarray(ln_img(inp["fn_s"], inp["fn_b"]))

    shared = {
        "cos_k": bf(cosT),
        "sin_k": bf(sinT),
        "memT_k": bf(memT_k),
        "mem_aug": bf(mem_aug_img),
        **{k: np.ascontiguousarray(v) for k, v in st.items()},
    }
    in_maps = []
    for c in range(NCORE):
        m = dict(shared)
        m["xT0"] = np.ascontiguousarray(xT0[:, c * S : (c + 1) * S])
        m["cos_q"] = bf(cosT[:, c * S : (c + 1) * S])
        m["sin_q"] = bf(sinT[:, c * S : (c + 1) * S])
        in_maps.append(m)
    return in_maps


# ----------------------------------------------------------------------------
# device kernel
# ----------------------------------------------------------------------------


def build_nc():
    nc = bacc.Bacc("TRN2", target_bir_lowering=False, debug=False, num_devices=NCORE)

    def din(name, shape, dt=F32):
        return nc.dram_tensor(name, list(shape), dt, kind="ExternalInput").ap()

    T = {}
    T["xT0"] = din("xT0", [D, S])
    T["cos_q"] = din("cos_q", [128, S], BF16)
    T["sin_q"] = din("sin_q", [128, S], BF16)
    T["cos_k"] = din("cos_k", [128, SEQ], BF16)
    T["sin_k"] = din("sin_k", [128, SEQ], BF16)
    T["memT_k"] = din("memT_k", [KV, NK], BF16)
    T["mem_aug"] = din("mem_aug", [128, 129 * (KV + 1)], BF16)
    for n in ["w_sa_q", "w_sa_k", "w_sa_v", "w_sa_o", "w_ca_q", "w_ca_o"]:
        T[n] = din(n, [L, 128, 512], BF16)
    T["w_ca_k"] = din("w_ca_k", [L, KV, 256], BF16)
    T["w_ca_v"] = din("w_ca_v", [L, KV, 256], BF16)
    T["w_ff1"] = din("w_ff1", [L, 128, 4096], BF16)
    T["w_ff2"] = din("w_ff2", [L, 128, 4096], BF16)
    for n in ["b_sa_q", "b_sa_k", "b_ca_q", "b_ca_k", "b_sa_o", "b_ca_o", "b_ff2"]:
        T[n] = din(n, [L, 128, 2])
    T["b_ff1"] = din("b_ff1", [L, 128, 16])
    T["b_ca_k_row"] = din("b_ca_k_row", [L, 1, 256])
    T["ln_all"] = din("ln_all", [L, 128, 12])
    T["ln_fn"] = din("ln_fn", [128, 4])

    T["out"] = nc.dram_tensor("outT", [D, S], F32, kind="ExternalOutput").ap()

    CCN = 2 * D * S  # bf16 elements per rank: k [256*512] + v [512*256]
    T["cc_in"] = nc.dram_tensor("cc_in", [CCN], BF16, kind="Internal").ap()
    T["cc_out"] = nc.dram_tensor(
        "cc_out", [NCORE * CCN], BF16, kind="Internal", addr_space="Shared"
    ).ap()
    if "nocc" in VARIANT:
        T["cc_out"] = nc.dram_tensor("cc_loc", [NCORE * CCN], BF16, kind="Internal").ap()

    with TileContext(nc) as tc:
        _emit(nc, tc, T)
    nc.compile()
    return nc


def _emit(nc, tc, T):
    import contextlib

    ctx = contextlib.ExitStack()
    const = ctx.enter_context(tc.tile_pool(name="const", bufs=1))
    wpool = ctx.enter_context(tc.tile_pool(name="wpool", bufs=2))
    act = ctx.enter_context(tc.tile_pool(name="act", bufs=2))
    small = ctx.enter_context(tc.tile_pool(name="small", bufs=4))
    rows = ctx.enter_context(tc.tile_pool(name="rows", bufs=8))
    pmm = ctx.enter_context(tc.tile_pool(name="pmm", bufs=2, space="PSUM"))
    pacc = ctx.enter_context(tc.tile_pool(name="pacc", bufs=2, space="PSUM"))
    prow = pmm

    V = nc.vector
    SC = nc.scalar
    TE = nc.tensor
    SY = nc.sync

    cosq = const.tile([128, S], BF16)
    sinq = const.tile([128, S], BF16)
    maug = const.tile([128, 129 * (KV + 1)], BF16)
    cosk = const.tile([128, SEQ], BF16)
    sink = const.tile([128, SEQ], BF16)
    SY.dma_start(out=cosq, in_=T["cos_q"])
    SY.dma_start(out=sinq, in_=T["sin_q"])
    SY.dma_start(out=cosk, in_=T["cos_k"])
    SY.dma_start(out=sink, in_=T["sin_k"])
    SY.dma_start(out=maug, in_=T["mem_aug"])
    ones_col_bf = const.tile([128, 1], BF16)
    V.memset(ones_col_bf, 1.0)
    ones_col_f = const.tile([128, 1], F32)
    V.memset(ones_col_f, 1.0)
    ones_col = const.tile([128, 1], F32R)
    V.tensor_copy(ones_col, ones_col_f)
    ones_row_f = const.tile([1, S], F32)
    V.memset(ones_row_f, 1.0)
    ones_row = const.tile([1, 128], F32R)
    V.tensor_copy(ones_row, ones_row_f[:, 0:128])
    ones_row_s = const.tile([1, S], F32R)
    V.tensor_copy(ones_row_s, ones_row_f)
    eps_row = const.tile([1, 1], F32)
    V.memset(eps_row, EPS)
    lnfn = const.tile([128, 4], F32)
    SY.dma_start(out=lnfn, in_=T["ln_fn"])

    x = act.tile([128, 2 * S], F32R, tag="x")
    SY.dma_start(out=x.rearrange("p (dh s) -> p dh s", dh=2),
                 in_=T["xT0"].bitcast(F32R).rearrange("(dh p) s -> p dh s", p=128))

    def f32r(ap):
        return ap.bitcast(F32R)

    def layer_norm(x_in, ln_tile, ln_off, out_dt=BF16):
        s1 = prow.tile([1, S], F32, tag="mm")
        xsq = act.tile([128, 2 * S], F32R, tag="xsq", bufs=1)
        SC.activation(out=xsq, in_=x_in, func=AF.Square)
        for dh in range(2):
            TE.matmul(s1, ones_col, x_in[:, dh * S : (dh + 1) * S],
                      start=(dh == 0), stop=(dh == 1))
        s2 = prow.tile([1, S], F32, tag="mm")
        for dh in range(2):
            TE.matmul(s2, ones_col, xsq[:, dh * S : (dh + 1) * S],
                      start=(dh == 0), stop=(dh == 1))
        m = rows.tile([1, S], F32R, tag="r")
        V.tensor_scalar_mul(m, s1, 1.0 / D)
        msq = rows.tile([1, S], F32, tag="r")
        V.tensor_tensor(msq, m, m, AL.mult)
        vr = rows.tile([1, S], F32, tag="r")
        V.scalar_tensor_tensor(out=vr, in0=s2, scalar=1.0 / D, in1=msq,
                               op0=AL.mult, op1=AL.subtract)
        lnv = rows.tile([1, S], F32, tag="r")
        SC.activation(out=lnv, in_=vr, func=AF.Ln, bias=eps_row)
        rstd = rows.tile([1, S], F32R, tag="r")
        SC.activation(out=rstd, in_=lnv, func=AF.Exp, scale=-0.5)

        bm_ps = pmm.tile([128, S], F32, tag="mm")
        TE.matmul(bm_ps, ones_row, m, start=True, stop=True)
        a_ps = pmm.tile([128, S], F32, tag="mm")
        TE.matmul(a_ps, ones_row, rstd, start=True, stop=True)
        t2 = act.tile([128, 2 * S], out_dt, tag="t2")
        for dh in range(2):
            u1 = small.tile([128, S], F32, tag="lntmp")
            V.tensor_tensor(u1, x_in[:, dh * S : (dh + 1) * S], bm_ps, AL.subtract)
            u2 = small.tile([128, S], F32, tag="lntmp")
            V.tensor_tensor(u2, u1, a_ps, AL.mult)
            V.tensor_scalar(out=t2[:, dh * S : (dh + 1) * S], in0=u2,
                            scalar1=ln_tile[:, ln_off + dh : ln_off + dh + 1],
                            scalar2=ln_tile[:, ln_off + 2 + dh : ln_off + 3 + dh],
                            op0=AL.mult, op1=AL.add)
        return t2

    GP = nc.gpsimd

    def rope(src_bf, cos_ap, sin_ap, dst_bf, n=S):
        e = src_bf[:, 0:n]
        o = src_bf[:, n : 2 * n]
        t1 = small.tile([128, S], BF16, tag="rtv")
        t2_ = small.tile([128, S], BF16, tag="rtv")
        V.tensor_tensor(t1[:, 0:n], e, cos_ap, AL.mult)
        V.tensor_tensor(t2_[:, 0:n], o, sin_ap, AL.mult)
        V.tensor_tensor(dst_bf[:, 0:n], t1[:, 0:n], t2_[:, 0:n], AL.subtract)
        t3 = small.tile([128, S], BF16, tag="rtv")
        t4 = small.tile([128, S], BF16, tag="rtv")
        V.tensor_tensor(t3[:, 0:n], e, sin_ap, AL.mult)
        V.tensor_tensor(t4[:, 0:n], o, cos_ap, AL.mult)
        V.tensor_tensor(dst_bf[:, n : 2 * n], t3[:, 0:n], t4[:, 0:n], AL.add)

    def proj_qk(t2, w_tile, b_tile, dst_bf):
        for dh in range(2):
            ps = pmm.tile([128, S], F32, tag="mm")
            for kh in range(2):
                TE.matmul(ps, w_tile[:, kh * 256 + dh * 128 : kh * 256 + dh * 128 + 128],
                          t2[:, kh * S : (kh + 1) * S],
                          start=(kh == 0), stop=(kh == 1))
            V.tensor_scalar_add(dst_bf[:, dh * S : (dh + 1) * S], ps,
                                b_tile[:, dh : dh + 1])

    def load_w(name, l, shape, dt=F32, bufs=None):
        t = wpool.tile(shape, dt, tag=name, bufs=bufs)
        SY.dma_start(out=t, in_=T[name][l])
        return t

    for l in [ll for _ in range(REPS) for ll in range(L)]:
        wsaq = load_w("w_sa_q", l, [128, 512], BF16)
        wsak = load_w("w_sa_k", l, [128, 512], BF16)
        wsav = load_w("w_sa_v", l, [128, 512], BF16)
        wsao = load_w("w_sa_o", l, [128, 512], BF16)
        wcaq = load_w("w_ca_q", l, [128, 512], BF16)
        wcak = load_w("w_ca_k", l, [KV, 256], BF16)
        wcav = load_w("w_ca_v", l, [KV, 256], BF16)
        wcao = load_w("w_ca_o", l, [128, 512], BF16)
        wff1 = load_w("w_ff1", l, [128, 4096], BF16, bufs=1)
        wff2 = load_w("w_ff2", l, [128, 4096], BF16, bufs=1)
        bsaq = load_w("b_sa_q", l, [128, 2])
        bsak = load_w("b_sa_k", l, [128, 2])
        bcaq = load_w("b_ca_q", l, [128, 2])
        bcak = load_w("b_ca_k", l, [128, 2])
        bsao = load_w("b_sa_o", l, [128, 2])
        bcao = load_w("b_ca_o", l, [128, 2])
        bff1 = load_w("b_ff1", l, [128, 16])
        bff2 = load_w("b_ff2", l, [128, 2])
        lnw = load_w("ln_all", l, [128, 12])

        # ---------- self attention ----------
        t2 = layer_norm(x, lnw, 0)
        qpre = act.tile([128, 2 * S], BF16, tag="qpre")
        proj_qk(t2, wsaq, bsaq, qpre)
        qr = act.tile([128, 2 * S], BF16, tag="qr")
        rope(qpre, cosq, sinq, qr)
        kpre = act.tile([128, 2 * S], BF16, tag="qpre")
        proj_qk(t2, wsak, bsak, kpre)
        kr = act.tile([128, 2 * S], BF16, tag="kr")
        rope(kpre, cosq, sinq, kr)

        vown = act.tile([128, 4 * 256], BF16, tag="vown")
        for tcb in range(4):
            ps = pmm.tile([128, 256], F32, tag="mm")
            for kh in range(2):
                TE.matmul(ps,
                          t2[:, kh * S + tcb * 128 : kh * S + tcb * 128 + 128],
                          wsav[:, kh * 256 : (kh + 1) * 256],
                          start=(kh == 0), stop=(kh == 1))
            V.tensor_copy(vown[:, tcb * 256 : (tcb + 1) * 256], ps)

        SY.dma_start(
            out=T["cc_in"][0 : D * S].rearrange("(dh p s) -> p dh s", p=128, s=S),
            in_=kr.rearrange("p (dh s) -> p dh s", dh=2),
        )
        SY.dma_start(
            out=T["cc_in"][D * S : 2 * D * S].rearrange(
                "(tc p o) -> p tc o", p=128, o=256
            ),
            in_=vown.rearrange("p (tc o) -> p tc o", o=256),
        )
        if "nocc" in VARIANT:
            # timing bisection only: fake the gather with local copies
            for r in range(NCORE):
                SY.dma_start(out=T["cc_out"][r * 2 * D * S : (r + 1) * 2 * D * S],
                             in_=T["cc_in"][:])
        else:
            nc.gpsimd.collective_compute(
                "AllGather",
                AL.bypass,
                replica_groups=[list(range(NCORE))],
                ins=[T["cc_in"][:]],
                outs=[T["cc_out"][:]],
            )
        av0 = pacc.tile([128, S], F32, tag="acc")
        av1 = pacc.tile([128, S], F32, tag="acc")
        den = prow.tile([1, S], F32, tag="mm")
        for kc2 in range(16):
            sc2 = pmm.tile([128, 2 * S], F32, tag="mm2", bufs=2)
            P2 = small.tile([128, 2 * S], BF16, tag="P", bufs=4)
            kchs = []
            vchs = []
            for h in range(2):
                kc = 2 * kc2 + h
                r_, tcq = kc // 4, kc % 4
                base = r_ * 2 * D * S
                kch = small.tile([128, 256], BF16, tag="kch", bufs=4)
                SY.dma_start(
                    out=kch.rearrange("p (dh c) -> p dh c", dh=2),
                    in_=T["cc_out"][base : base + D * S].rearrange(
                        "(dh p s) -> p dh s", p=128, s=S
                    )[:, :, tcq * 128 : (tcq + 1) * 128],
                )
                vch = small.tile([128, 256], BF16, tag="vch", bufs=4)
                SY.dma_start(
                    out=vch,
                    in_=T["cc_out"][
                        base + D * S + tcq * 128 * 256 : base + D * S
                        + (tcq + 1) * 128 * 256
                    ].rearrange("(p o) -> p o", p=128),
                )
                kchs.append(kch)
                vchs.append(vch)
                for dh in range(2):
                    TE.matmul(sc2[:, h * S : (h + 1) * S],
                              kch[:, dh * 128 : (dh + 1) * 128],
                              qr[:, dh * S : (dh + 1) * S],
                              start=(dh == 0), stop=(dh == 1))
            SC.activation(out=P2, in_=sc2, func=AF.Exp, scale=1.0 / 16.0)
            for h in range(2):
                kc = 2 * kc2 + h
                Ph = P2[:, h * S : (h + 1) * S]
                TE.matmul(den, ones_col_bf, Ph, start=(kc == 0), stop=(kc == 31))
                TE.matmul(av0, vchs[h][:, 0:128], Ph,
                          start=(kc == 0), stop=(kc == 31))
                TE.matmul(av1, vchs[h][:, 128:256], Ph,
                          start=(kc == 0), stop=(kc == 31))

        lnden = rows.tile([1, S], F32, tag="r")
        SC.activation(out=lnden, in_=den, func=AF.Ln)
        recip = rows.tile([1, S], F32R, tag="r")
        SC.activation(out=recip, in_=lnden, func=AF.Exp, scale=-1.0)
        bc = pmm.tile([128, S], F32, tag="mm")
        TE.matmul(bc, ones_row, recip, start=True, stop=True)
        bcs = small.tile([128, S], F32, tag="bcs", bufs=2)
        V.tensor_copy(bcs, bc)
        oin = act.tile([128, 2 * S], BF16, tag="oin")
        V.tensor_tensor(oin[:, 0:S], av0, bcs, AL.mult)
        V.tensor_tensor(oin[:, S : 2 * S], av1, bcs, AL.mult)

        xn = act.tile([128, 2 * S], F32R, tag="x")
        for dh in range(2):
            ps = pmm.tile([128, S], F32, tag="mm")
            for kh in range(2):
                TE.matmul(ps, wsao[:, kh * 256 + dh * 128 : kh * 256 + dh * 128 + 128],
                          oin[:, kh * S : (kh + 1) * S],
                          start=(kh == 0), stop=(kh == 1))
            V.scalar_tensor_tensor(out=xn[:, dh * S : (dh + 1) * S], in0=ps,
                                   scalar=bsao[:, dh : dh + 1],
                                   in1=x[:, dh * S : (dh + 1) * S],
                                   op0=AL.add, op1=AL.add)
        x = xn

        # ---------- cross attention ----------
        t2c = layer_norm(x, lnw, 4)
        qpre2 = act.tile([128, 2 * S], BF16, tag="qpre")
        proj_qk(t2c, wcaq, bcaq, qpre2)
        qrc = act.tile([128, 2 * S], BF16, tag="qr")
        rope(qpre2, cosq, sinq, qrc)

        U = pacc.tile([128, S], F32, tag="acc")
        n_mm_u = NSPC * 4 + 1
        mmu = 0
        for scc in range(NSPC):
            koff = scc * 512
            coff = (scc % 8) * 512
            mkc = small.tile([KV, 512], BF16, tag="mkc", bufs=3)
            SY.dma_start(out=mkc, in_=T["memT_k"][:, koff : koff + 512])
            kme = small.tile([128, 2 * 512], BF16, tag="kme", bufs=3)
            for dh in range(2):
                kp = pmm.tile([128, 512], F32, tag="mm")
                TE.matmul(kp, wcak[:, dh * 128 : (dh + 1) * 128], mkc,
                          start=True, stop=True)
                V.tensor_scalar_add(kme[:, dh * 512 : (dh + 1) * 512], kp,
                                    bcak[:, dh : dh + 1])
            krm = small.tile([128, 2 * 512], BF16, tag="krm", bufs=4)
            rope(kme, cosk[:, coff : coff + 512], sink[:, coff : coff + 512],
                 krm, n=512)
            for jp in range(2):
                sc2 = pmm.tile([128, 2 * S], F32, tag="mm2", bufs=2)
                P2 = small.tile([128, 2 * S], BF16, tag="P", bufs=4)
                for h in range(2):
                    j = 2 * jp + h
                    for dh in range(2):
                        TE.matmul(sc2[:, h * S : (h + 1) * S],
                                  krm[:, dh * 512 + j * 128 : dh * 512 + j * 128 + 128],
                                  qrc[:, dh * S : (dh + 1) * S],
                                  start=(dh == 0), stop=(dh == 1))
                SC.activation(out=P2, in_=sc2, func=AF.Exp, scale=1.0 / 16.0)
                for h in range(2):
                    ci = scc * 4 + 2 * jp + h
                    TE.matmul(U[0:65, :], maug[:, ci * 65 : (ci + 1) * 65],
                              P2[:, h * S : (h + 1) * S],
                              start=(mmu == 0), stop=(mmu == n_mm_u - 1))
                    mmu += 1
        # object pointer chunk (64 keys, no rope)
        mko = small.tile([KV, 64], BF16, tag="mko", bufs=1)
        SY.dma_start(out=mko, in_=T["memT_k"][:, NSP:NK])
        kobj = small.tile([128, 2 * 64], BF16, tag="kobj", bufs=1)
        for dh in range(2):
            kp = pmm.tile([128, 64], F32, tag="mm")
            TE.matmul(kp, wcak[:, dh * 128 : (dh + 1) * 128], mko,
                      start=True, stop=True)
            V.tensor_scalar_add(kobj[:, dh * 64 : (dh + 1) * 64], kp,
                                bcak[:, dh : dh + 1])
        sco = pmm.tile([64, S], F32, tag="mm")
        for dh in range(2):
            TE.matmul(sco, kobj[:, dh * 64 : (dh + 1) * 64],
                      qrc[:, dh * S : (dh + 1) * S],
                      start=(dh == 0), stop=(dh == 1))
        Po = small.tile([64, S], BF16, tag="Po", bufs=1)
        SC.activation(out=Po, in_=sco, func=AF.Exp, scale=1.0 / 16.0)
        TE.matmul(U[0:65, :], maug[0:64, 128 * 65 : 128 * 65 + 65], Po,
                  start=False, stop=True)

        lnd2 = rows.tile([1, S], F32, tag="r")
        SC.activation(out=lnd2, in_=U[64:65, :], func=AF.Ln)
        rec2 = rows.tile([1, S], F32R, tag="r")
        SC.activation(out=rec2, in_=lnd2, func=AF.Exp, scale=-1.0)
        bc2 = pmm.tile([128, S], F32, tag="mm")
        TE.matmul(bc2, ones_row, rec2, start=True, stop=True)
        bcs2 = small.tile([128, S], F32, tag="bcs", bufs=2)
        V.tensor_copy(bcs2, bc2)
        un = small.tile([64, S], BF16, tag="un", bufs=2)
        V.tensor_tensor(un, U[0:64, :], bcs2[0:64, :], AL.mult)

        oin2 = act.tile([128, 2 * S], BF16, tag="oin")
        for dh in range(2):
            ps = pmm.tile([128, S], F32, tag="mm")
            TE.matmul(ps, wcav[:, dh * 128 : (dh + 1) * 128], un,
                      start=True, stop=True)
            V.tensor_copy(oin2[:, dh * S : (dh + 1) * S], ps)
        xn2 = act.tile([128, 2 * S], F32R, tag="x")
        for dh in range(2):
            ps = pmm.tile([128, S], F32, tag="mm")
            for kh in range(2):
                TE.matmul(ps, wcao[:, kh * 256 + dh * 128 : kh * 256 + dh * 128 + 128],
                          oin2[:, kh * S : (kh + 1) * S],
                          start=(kh == 0), stop=(kh == 1))
            V.scalar_tensor_tensor(out=xn2[:, dh * S : (dh + 1) * S], in0=ps,
                                   scalar=bcao[:, dh : dh + 1],
                                   in1=x[:, dh * S : (dh + 1) * S],
                                   op0=AL.add, op1=AL.add)
        x = xn2

        # ---------- FFN ----------
        t3 = layer_norm(x, lnw, 8)
        hT = act.tile([128, 16 * S], BF16, tag="hT", bufs=1)
        for mc in range(16):
            ps = pmm.tile([128, S], F32, tag="mm")
            for kh in range(2):
                TE.matmul(ps,
                          wff1[:, kh * 2048 + mc * 128 : kh * 2048 + mc * 128 + 128],
                          t3[:, kh * S : (kh + 1) * S],
                          start=(kh == 0), stop=(kh == 1))
            V.tensor_scalar(out=hT[:, mc * S : (mc + 1) * S], in0=ps,
                            scalar1=bff1[:, mc : mc + 1], scalar2=0.0,
                            op0=AL.add, op1=AL.max)
        xn3 = act.tile([128, 2 * S], F32R, tag="x")
        for dh in range(2):
            ps = pmm.tile([128, S], F32, tag="mm")
            for kh in range(16):
                TE.matmul(ps, wff2[:, kh * 256 + dh * 128 : kh * 256 + dh * 128 + 128],
                          hT[:, kh * S : (kh + 1) * S],
                          start=(kh == 0), stop=(kh == 15))
            V.scalar_tensor_tensor(out=xn3[:, dh * S : (dh + 1) * S], in0=ps,
                                   scalar=bff2[:, dh : dh + 1],
                                   in1=x[:, dh * S : (dh + 1) * S],
                                   op0=AL.add, op1=AL.add)
        x = xn3

    xf = layer_norm(x, lnfn, 0, out_dt=F32)
    SY.dma_start(out=T["out"].rearrange("(dh p) s -> p dh s", p=128),
                 in_=xf.rearrange("p (dh s) -> p dh s", dh=2))
    ctx.close()


# ----------------------------------------------------------------------------
# entry point
# ----------------------------------------------------------------------------

_NC_CACHE = {}


def _get_nc():
    if "nc" not in _NC_CACHE:
        _NC_CACHE["nc"] = build_nc()
    return _NC_CACHE["nc"]


def kernel(**inputs):
    nc = _get_nc()
    in_maps = prepare_inputs(inputs)
    res = run_bass_kernel_spmd(nc, in_maps, core_ids=list(range(NCORE)))
    outs = [np.asarray(res.results[c]["outT"]) for c in range(NCORE)]
    X = np.concatenate(outs, axis=1)  # [256, 4096] in permuted-d space
    Xo = np.empty_like(X)
    Xo[PERM] = X
    return np.ascontiguousarray(Xo.reshape(1, D, FEAT, FEAT)).astype(np.float32)
